# revision 15
# baseline (speedup 1.0000x reference)
"""S4D "CopyingModel" Trainium2 Bass kernel (v6 — DMA rings + scan/drain
scheduling for continuous PE).

Math: logits = (s4d_scan(emb[x]) + emb[x]*D) @ W_out + b_out, with a
per-channel diagonal SSM (d_model=1024 channels, d_state=64).

Strategy (8 NeuronCores, channel-sharded: 128 channels per core).
Tiles are b-major (tile t = b*32 + c); every PE stream is contiguous:
  - u_sb [j, d, t]: per-channel matmul rhs contiguous
  - x_sb [p, b, q, s]: chunk-carry tensor_tensor_scan contiguous per b
  - y_sb [j, tg, d, ti] (t = 4*tg + ti): drain writes in runs of 4,
    transpose LDW reads at 8B stride (cheap on both sides)
v6 changes vs v5:
  - one-hot DMAs ride the GpSimd HWDGE ring, weights ride the Sync
    ring: the two streams no longer serialize behind each other
  - one-hot groups doubled (GA=16 -> 4KB per partition row), deep
    prefetch (bufs=6) starting before identity/memset setup
  - chunk-carry scans are q-block-major: one tensor_tensor_scan per
    phase-B channel group (all batches at once, p0 broadcast over b),
    so phase D's Cb matmuls never wait on a late scan
  - B/D PSUM drains on Scalar+GpSimd (DVE stays free for scans);
    A/E drains on Vector+Scalar
"""

import os
from contextlib import ExitStack

import numpy as np

BATCH = 8
SEQ = 4096
D_MODEL = 1024
N_STATE = 64
VOCAB = 64
L = 128                   # chunk length
NCH = SEQ // L            # 32 chunks
NCORES = 8
DPC = D_MODEL // NCORES   # 128 channels per core
BC = NCH * BATCH          # 256 token tiles; tile t = b*NCH + c

GA = 8    # one-hot tiles per DMA (phase A)
GE = 16   # E channels per DMA
GT = 8    # T channels per DMA
GQ = 16   # Cb channel-pairs per DMA

LAST_RESULTS = None       # BassKernelResults of the most recent run


def _precompute_host(emb, log_neg_A, Bmat, C, Dvec, log_dt):
    """Float64 host precompute of all device operands."""
    dt = np.exp(log_dt.astype(np.float64))                    # (D,)
    A = -np.exp(log_neg_A.astype(np.float64))                 # (D,N)
    dA = np.exp(dt[:, None] * A)                              # (D,N)
    dB = (dA - 1.0) / A * Bmat.astype(np.float64)             # (D,N)
    w = C.astype(np.float64) * dB                             # (D,N)

    # dApow[d,n,k] = dA^k, k=0..L-1
    dApow = np.ones((D_MODEL, N_STATE, L))
    np.cumprod(np.broadcast_to(dA[:, :, None], (D_MODEL, N_STATE, L - 1)),
               axis=2, out=dApow[:, :, 1:])
    K = np.einsum("dn,dnk->dk", w, dApow)                     # (D,L)
    K[:, 0] += Dvec.astype(np.float64)                        # fold skip

    # Toeplitz lhsT: T[d][j,i] = K[d, i-j] for i>=j
    T = np.zeros((D_MODEL, L, L), np.float32)
    Kf = K.astype(np.float32)
    for k in range(L):
        idx = np.arange(L - k)
        T[:, idx, idx + k] = Kf[:, k][:, None]

    # E lhsT [d, j, n] = dA^(L-1-j) * dB
    E = (dApow[:, :, ::-1] * dB[:, :, None]).transpose(0, 2, 1)  # (D,L,N)
    # Cb lhsT [d, n, i] = C * dA^(i+1)
    dApow1 = dApow * dA[:, :, None]
    Cb = C.astype(np.float64)[:, :, None] * dApow1               # (D,N,L)
    P = dApow1[:, :, L - 1]                                      # dA^L (D,N)
    return (T.astype(np.float16), E.astype(np.float16),
            Cb.astype(np.float16), P)


def _emit_kernel(nc, tile, mybir, make_identity):
    f16 = mybir.dt.float16
    f32 = mybir.dt.float32

    f8 = mybir.dt.float8e4
    onehotT = nc.dram_tensor("onehot_t", [VOCAB // 2, 2, BC * L], f8,
                             kind="ExternalInput").ap()
    embs_hi = nc.dram_tensor("emb_hi", [VOCAB // 2, 2, DPC], f8,
                             kind="ExternalInput").ap()
    embs_lo = nc.dram_tensor("emb_lo", [VOCAB // 2, 2, DPC], f8,
                             kind="ExternalInput").ap()
    # [group, j, ch_in_group, i]
    t_all = nc.dram_tensor("t_all", [DPC // GT, L, GT, L], f16,
                           kind="ExternalInput").ap()
    e_all = nc.dram_tensor("e_all", [DPC // GE, L, GE, N_STATE], f16,
                           kind="ExternalInput").ap()
    # [group, p=(par,n), q_in_group, i]
    cb_all = nc.dram_tensor("cb_all", [64 // GQ, 128, GQ, L], f16,
                            kind="ExternalInput").ap()
    # scan multiplier: [p=(par,n), q, s]; 0 at s=0 (segment reset)
    p0 = nc.dram_tensor("p0", [128, 64, NCH], f16, kind="ExternalInput").ap()
    w2 = nc.dram_tensor("w2", [DPC, VOCAB], f16, kind="ExternalInput").ap()
    # [G, p=(ph,v), s, i*L]  (see host unpack)
    out_t = nc.dram_tensor("out_t", [BC // 8, 128, 2, 2 * L], f16,
                           kind="ExternalOutput").ap()

    with tile.TileContext(nc) as tc, ExitStack() as ctx:
        persist = ctx.enter_context(tc.tile_pool(name="persist", bufs=1))
        u_sb = persist.tile([128, DPC, BC], f16, name="u_sb")    # [j, d, t]
        # y: [j, tg, d, ti], t = 4*tg + ti
        y_sb = persist.tile([128, BC // 4, DPC, 4], f16, name="y_sb")
        # scan buffer [p=(par,n), b, q, s]; B writes S[c] into s=c+1,
        # slot 0 is zero; after the scan slot s=c holds hstart[c]
        x_sb = persist.tile([128, BATCH, 64, NCH], f16, name="x_sb")
        p0_sb = persist.tile([128, 64, NCH], f16, name="p0_sb")
        emb_hi_sb = persist.tile([VOCAB // 2, 2, DPC], f8, name="emb_hi_sb")
        emb_lo_sb = persist.tile([VOCAB // 2, 2, DPC], f8, name="emb_lo_sb")
        w2_sb = persist.tile([DPC, VOCAB], f16, name="w2_sb")
        ident = persist.tile([128, 128], f16, name="ident")

        # one-hot staging pool opened first; DMAs ride the GpSimd ring
        ohp = ctx.enter_context(tc.tile_pool(name="ohp", bufs=7))
        oh_ts = {}

        def fetch_oh(g):
            oh_ts[g] = ohp.tile([VOCAB // 2, 2, GA * L], f8, name="oh_w",
                                tag="oh_w")
            nc.gpsimd.dma_start(
                out=oh_ts[g], in_=onehotT[:, :, g * GA * L:(g + 1) * GA * L])

        # first bytes on the wire before any setup compute
        nc.sync.dma_start(out=emb_hi_sb, in_=embs_hi)
        nc.sync.dma_start(out=emb_lo_sb, in_=embs_lo)
        for g in range(5):
            fetch_oh(g)

        # weight pools (Sync ring, independent of the one-hot stream)
        ewp = ctx.enter_context(tc.tile_pool(name="ewp", bufs=4))
        twp = ctx.enter_context(tc.tile_pool(name="twp", bufs=3))
        cbp = ctx.enter_context(tc.tile_pool(name="cbp", bufs=2))
        e_ws, t_ws, cb_ws = {}, {}, {}

        def fetch_e(g):
            e_ws[g] = ewp.tile([L, GE, N_STATE], f16, name="e_w", tag="e_w")
            nc.sync.dma_start(out=e_ws[g], in_=e_all[g])

        def fetch_t(g):
            t_ws[g] = twp.tile([L, GT, L], f16, name="t_w", tag="t_w")
            nc.sync.dma_start(out=t_ws[g], in_=t_all[g])

        def fetch_cb(g):
            cb_ws[g] = cbp.tile([128, GQ, L], f16, name="cb_w", tag="cb_w")
            nc.sync.dma_start(out=cb_ws[g], in_=cb_all[g])

        # weight prefetch: e first (phase B), then t (phase D), then cb
        for g in range(4):
            fetch_e(g)
        nc.sync.dma_start(out=p0_sb, in_=p0)
        nc.sync.dma_start(out=w2_sb, in_=w2)
        for g in range(2):
            fetch_t(g)
        fetch_cb(0)

        make_identity(nc, ident)
        nc.vector.memset(x_sb[:, :, :, 0], 0.0)

        def cp(i, out, in_):
            if i % 2 == 0:
                nc.vector.tensor_copy(out, in_)
            else:
                nc.scalar.copy(out, in_)

        def cp3(i, out, in_):
            # gpsimd cannot read PSUM; scalar-heavy split, DVE does scans
            if i % 6 < 5:
                nc.scalar.copy(out, in_)
            else:
                nc.vector.tensor_copy(out, in_)

        # ---- Phase A: embedding (one-hot @ emb slice) -> u_sb ----
        with tc.tile_pool(name="ps_a", bufs=3, space="PSUM") as ps_a:
            for g in range(BC // GA):                    # 32 groups of 8
                if g + 5 < BC // GA:
                    fetch_oh(g + 5)
                elif g == BC // GA - 5:
                    # tail of the fetch schedule: remaining t groups
                    fetch_t(2)
                    fetch_t(3)
                oh = oh_ts[g]
                ups = ps_a.tile([128, 8, DPC], f32)      # 2 PSUM banks
                for i in range(8):
                    # one accumulation group per 2KB zero region; each
                    # slot takes a DoubleRow fp8 hi + lo pair
                    lh = oh[:, :, i * L:(i + 1) * L]
                    nc.tensor.matmul(ups[:, i, :], lhsT=lh, rhs=emb_hi_sb,
                                     start=(i % 4 == 0), stop=False,
                                     perf_mode=mybir.MatmulPerfMode.DoubleRow)
                    nc.tensor.matmul(ups[:, i, :], lhsT=lh, rhs=emb_lo_sb,
                                     start=False, stop=(i % 4 == 3),
                                     perf_mode=mybir.MatmulPerfMode.DoubleRow)
                t0 = g * GA
                # transposing drain: strided fp32 PSUM reads (cheap),
                # contiguous f16 writes into u_sb[:, d, t]
                cp(g, u_sb[:, :, t0:t0 + 8], ups.transpose([0, 2, 1]))

        # ---- Phase B: chunk-end states S[c] -> x_sb slots 1..31 ----
        # After each group's drain, scan that q-block for all batches
        # (DVE), so phase D's Cb matmuls never wait.
        with tc.tile_pool(name="ps_s", bufs=4, space="PSUM") as ps_s:
            for g in range(DPC // GE):                   # 8 groups of 16 ch
                if g >= 4:
                    fetch_e(g)
                e_w = e_ws[g]
                for k in range(GE // 4):                 # 2 q-pairs per bank
                    qb = (g * GE) // 2 + 2 * k           # first q of the bank
                    s_ps = ps_s.tile([128, 2, BATCH, NCH], f32)
                    for jq in range(2):
                        q = qb + jq
                        for par in range(2):
                            dl = 2 * q + par
                            # per-partition-range groups; the sim's group
                            # check mis-addresses split groups, skip it
                            nc.tensor.matmul(
                                s_ps[64 * par:64 * (par + 1), jq, :, :],
                                lhsT=e_w[:, dl - g * GE, :], rhs=u_sb[:, dl, :],
                                start=(jq == 0), stop=(jq == 1),
                                skip_group_check=True,
                                tile_position=(0, 64 * par) if par else None)
                    # drain S[c] -> x_sb[:, b, q, c+1] ((q2,b,c) -> (b,q2,s))
                    out_ap = x_sb[:, :, qb:qb + 2, 1:NCH].transpose([0, 2, 1, 3])
                    (nc.vector.tensor_copy if (k + g) % 4 == 3 else nc.scalar.copy)(out_ap, s_ps[:, :, :, 0:NCH - 1])
                if g % 2 == 1:
                    # scan the finished q-quarter for every batch, split
                    # DVE / GpSimd so neither engine becomes critical
                    q0 = (g // 2) * 16
                    mul = p0_sb[:, q0:q0 + 16, :].rearrange("p q s -> p (q s)")
                    for b in range(BATCH):
                        seg = x_sb[:, b, q0:q0 + 16, :].rearrange(
                            "p q s -> p (q s)")
                        nc.vector.tensor_tensor_scan(
                            out=seg, data0=mul, data1=seg,
                            initial=0.0, op0=mybir.AluOpType.mult,
                            op1=mybir.AluOpType.add)

        # ---- Phase D: per-channel y = T^T u (+) Cb^T hstart -> y_sb ----
        with tc.tile_pool(name="ps_y", bufs=4, space="PSUM") as ps_y:
            for q in range(64):
                if q % GQ == 0 and q > 0:
                    fetch_cb(q // GQ)
                if q % (GT // 2) == 0 and q >= 16:   # t0..t3 prefetched in A
                    fetch_t((2 * q) // GT)
                t_w = t_ws[(2 * q) // GT]
                cb_w = cb_ws[q // GQ]
                # one full 2KB bank per channel (row-offset tile_position
                # matmuls crash when a bank holds two accumulation slots),
                # but pack the q-pair into one 2-bank tile so the drain is
                # a single instruction
                y_ps = ps_y.tile([L, 2, 2 * BC], f32, name="y_ps",
                                 tag="y_ps")
                for par in range(2):
                    dl = 2 * q + par
                    nc.tensor.matmul(y_ps[:, par, 0:BC],
                                     lhsT=t_w[:, dl % GT, :],
                                     rhs=u_sb[:, dl, :],
                                     start=True, stop=False)
                for par in range(2):
                    h = x_sb[64 * par:64 * (par + 1), :, q, :]   # [64, b, c]
                    nc.tensor.matmul(
                        y_ps[:, par, 0:BC],
                        lhsT=cb_w[64 * par:64 * (par + 1), q % GQ, :],
                        rhs=h, start=False, stop=True,
                        tile_position=(64 * par, 0) if par else None)
                # [j, 2, (tg ti)] -> y_sb[:, tg, 2q:2q+2, ti]
                dcp = (nc.scalar.copy if (q < 32 or q % 2 == 0)
                       else nc.vector.tensor_copy)
                dcp(y_sb[:, :, 2 * q:2 * q + 2, :],
                    y_ps[:, :, 0:BC].rearrange("p c (g t) -> p c g t",
                                               g=BC // 4)
                    .transpose([0, 2, 1, 3]))

        # ---- Phase E: transpose y + output projection -> out_t ----
        with tc.tile_pool(name="ytp", bufs=2) as ytp, \
             tc.tile_pool(name="lop", bufs=2) as lop, \
             tc.tile_pool(name="ps_t", bufs=3, space="PSUM") as ps_t, \
             tc.tile_pool(name="ps_o", bufs=4, space="PSUM") as ps_o:
            for G in range(BC // 8):                     # 32 super-groups
                op = ps_o.tile([128, 2, 2 * L], f32)     # 4 logit tiles/bank
                tp = ps_t.tile([128, 8, 128], f16)       # one full bank
                yt = ytp.tile([128, 8, 128], f16)
                for s in range(2):
                    for i in range(4):
                        # tile t = 8G+4s+i = 4*(2G+s) + i
                        nc.tensor.matmul(
                            tp[:, 4 * s + i, :], lhsT=y_sb[:, 2 * G + s, :, i],
                            rhs=ident, is_transpose=True,
                            start=(4 * s + i == 0), stop=(4 * s + i == 7))
                nc.vector.tensor_copy(yt[:, 0:4, :], tp[:, 0:4, :])
                nc.scalar.copy(yt[:, 4:8, :], tp[:, 4:8, :])
                for s in range(2):
                    for ph in range(2):
                        nc.tensor.matmul(
                            op[64 * ph:64 * (ph + 1), s, :],
                            lhsT=w2_sb,
                            rhs=yt[:, 4 * s + 2 * ph:4 * s + 2 * ph + 2, :],
                            start=(s == 0), stop=(s == 1),
                            skip_group_check=True,
                            tile_position=(0, 64 * ph) if ph else None)
                lo = lop.tile([128, 2, 2 * L], f16)
                cp(G, lo, op)
                nc.sync.dma_start(out=out_t[G], in_=lo)


def _build_nc():
    import concourse.tile as tile
    from concourse import bacc, mybir
    from concourse.masks import make_identity

    nc = bacc.Bacc(trn_type="TRN2", target_bir_lowering=False, debug=False)
    _emit_kernel(nc, tile, mybir, make_identity)
    nc.compile()
    return nc


_NC_CACHE = None


def kernel(x, emb, log_neg_A, B, C, D, log_dt, W_out, b_out):
    global LAST_RESULTS, _NC_CACHE
    from concourse.bass_utils import run_bass_kernel_spmd

    x = np.asarray(x).astype(np.int64)
    emb = np.asarray(emb, np.float32)
    log_neg_A = np.asarray(log_neg_A, np.float32)
    B_in = np.asarray(B, np.float32)
    C = np.asarray(C, np.float32)
    D_in = np.asarray(D, np.float32)
    log_dt = np.asarray(log_dt, np.float32)
    W_out = np.asarray(W_out, np.float32)
    b_out = np.asarray(b_out, np.float32)

    T, E, Cb, P = _precompute_host(emb, log_neg_A, B_in, C, D_in, log_dt)

    import ml_dtypes
    f8 = ml_dtypes.float8_e4m3fn
    # one-hot; b-major tiles: tok = (b*NCH + c)*L + j = row-major flat x;
    # fp8 DoubleRow packing [v//2, 2, tok]
    toks = x.reshape(-1)
    onehotT = (np.arange(VOCAB)[:, None] == toks[None, :]).astype(f8)
    onehotT = onehotT.reshape(VOCAB // 2, 2, BC * L)

    in_maps = []
    for core in range(NCORES):
        ds = slice(core * DPC, (core + 1) * DPC)
        # p0 layout [p=(par,n), q, s]: p = 64*par + n, d = 2*q + par; 0 at s=0
        Pc = P[ds].reshape(64, 2, N_STATE).transpose(1, 2, 0).reshape(128, 64)
        P0 = np.broadcast_to(Pc[:, :, None], (128, 64, NCH)).copy()
        P0[:, :, 0] = 0.0
        # t_all: [DPC,L,L] -> [DPC/GT, L, GT, L]
        Tc = np.ascontiguousarray(
            T[ds].reshape(DPC // GT, GT, L, L).transpose(0, 2, 1, 3))
        # e_all: [DPC,L,N] -> [DPC/GE, L, GE, N]
        Ec = np.ascontiguousarray(
            E[ds].reshape(DPC // GE, GE, L, N_STATE).transpose(0, 2, 1, 3))
        # cb_all: [DPC,N,L] -> pair-pack [64, 128=(par,n), L] -> groups of GQ
        Cbp = Cb[ds].reshape(64, 2 * N_STATE, L)   # [q, (par,n), L]
        Cbc = np.ascontiguousarray(
            Cbp.reshape(64 // GQ, GQ, 128, L).transpose(0, 2, 1, 3))
        emb_c = np.ascontiguousarray(emb[:, ds]).astype(np.float32)
        emb_hi = emb_c.astype(f8)
        emb_lo = (emb_c - emb_hi.astype(np.float32)).astype(f8)
        in_maps.append({
            "onehot_t": onehotT,
            "emb_hi": emb_hi.reshape(VOCAB // 2, 2, DPC),
            "emb_lo": emb_lo.reshape(VOCAB // 2, 2, DPC),
            "t_all": Tc,
            "e_all": Ec,
            "cb_all": Cbc,
            "p0": P0.astype(np.float16),
            "w2": np.ascontiguousarray(W_out[ds]).astype(np.float16),
        })

    if _NC_CACHE is None:
        _NC_CACHE = _build_nc()
    nc = _NC_CACHE

    trace = bool(int(os.environ.get("BASS_TRACE", "0") or "0"))
    LAST_RESULTS = run_bass_kernel_spmd(
        nc, in_maps, core_ids=list(range(NCORES)), trace=trace)

    # out_t[G, 64*ph+v, s, 128*i+j] = logitsT[v, (8G+4s+2ph+i)*128 + j]
    logitsT = np.zeros((VOCAB, BC * L), np.float64)
    for r in LAST_RESULTS.results:
        o = r["out_t"].astype(np.float64).reshape(32, 2, 64, 2, 2, L)
        logitsT += o.transpose(2, 0, 3, 1, 4, 5).reshape(VOCAB, BC * L)
    # b-major tiles: col = ((b*NCH + c)*L + j)
    out = logitsT.T.reshape(BATCH, SEQ, VOCAB)
    return (out + b_out.astype(np.float64)).astype(np.float32)


# revision 17
# speedup vs baseline: 1.2028x; 1.2028x over previous
"""S4D "CopyingModel" Trainium2 Bass kernel (v6 — DMA rings + scan/drain
scheduling for continuous PE).

Math: logits = (s4d_scan(emb[x]) + emb[x]*D) @ W_out + b_out, with a
per-channel diagonal SSM (d_model=1024 channels, d_state=64).

Strategy (8 NeuronCores, channel-sharded: 128 channels per core).
Tiles are b-major (tile t = b*32 + c); every PE stream is contiguous:
  - u_sb [j, d, t]: per-channel matmul rhs contiguous
  - x_sb [p, b, q, s]: chunk-carry tensor_tensor_scan contiguous per b
  - y_sb [j, tg, d, ti] (t = 4*tg + ti): drain writes in runs of 4,
    transpose LDW reads at 8B stride (cheap on both sides)
v6 changes vs v5:
  - one-hot DMAs ride the GpSimd HWDGE ring, weights ride the Sync
    ring: the two streams no longer serialize behind each other
  - one-hot groups doubled (GA=16 -> 4KB per partition row), deep
    prefetch (bufs=6) starting before identity/memset setup
  - chunk-carry scans are q-block-major: one tensor_tensor_scan per
    phase-B channel group (all batches at once, p0 broadcast over b),
    so phase D's Cb matmuls never wait on a late scan
  - B/D PSUM drains on Scalar+GpSimd (DVE stays free for scans);
    A/E drains on Vector+Scalar
"""

import os
from contextlib import ExitStack

import numpy as np

BATCH = 8
SEQ = 4096
D_MODEL = 1024
N_STATE = 64
VOCAB = 64
L = 128                   # chunk length
NCH = SEQ // L            # 32 chunks
NCORES = 8
DPC = D_MODEL // NCORES   # 128 channels per core
BC = NCH * BATCH          # 256 token tiles; tile t = b*NCH + c

GA = 8    # one-hot tiles per DMA (phase A)
GE = 16   # E channels per DMA
GT = 8    # T channels per DMA
GQ = 16   # Cb channel-pairs per DMA

LAST_RESULTS = None       # BassKernelResults of the most recent run


def _precompute_host(emb, log_neg_A, Bmat, C, Dvec, log_dt):
    """Float64 host precompute of all device operands."""
    dt = np.exp(log_dt.astype(np.float64))                    # (D,)
    A = -np.exp(log_neg_A.astype(np.float64))                 # (D,N)
    dA = np.exp(dt[:, None] * A)                              # (D,N)
    dB = (dA - 1.0) / A * Bmat.astype(np.float64)             # (D,N)
    w = C.astype(np.float64) * dB                             # (D,N)

    # dApow[d,n,k] = dA^k, k=0..L-1
    dApow = np.ones((D_MODEL, N_STATE, L))
    np.cumprod(np.broadcast_to(dA[:, :, None], (D_MODEL, N_STATE, L - 1)),
               axis=2, out=dApow[:, :, 1:])
    K = np.einsum("dn,dnk->dk", w, dApow)                     # (D,L)
    K[:, 0] += Dvec.astype(np.float64)                        # fold skip

    # Toeplitz lhsT: T[d][j,i] = K[d, i-j] for i>=j
    T = np.zeros((D_MODEL, L, L), np.float32)
    Kf = K.astype(np.float32)
    for k in range(L):
        idx = np.arange(L - k)
        T[:, idx, idx + k] = Kf[:, k][:, None]

    # E lhsT [d, j, n] = dA^(L-1-j) * dB
    E = (dApow[:, :, ::-1] * dB[:, :, None]).transpose(0, 2, 1)  # (D,L,N)
    # Cb lhsT [d, n, i] = C * dA^(i+1)
    dApow1 = dApow * dA[:, :, None]
    Cb = C.astype(np.float64)[:, :, None] * dApow1               # (D,N,L)
    P = dApow1[:, :, L - 1]                                      # dA^L (D,N)
    return (T.astype(np.float16), E.astype(np.float16),
            Cb.astype(np.float16), P)


def _emit_kernel(nc, tile, mybir, make_identity):
    f16 = mybir.dt.float16
    f32 = mybir.dt.float32

    onehotT = nc.dram_tensor("onehot_t", [VOCAB, BC * L], f16,
                             kind="ExternalInput").ap()
    embs = nc.dram_tensor("emb_s", [VOCAB, DPC], f16, kind="ExternalInput").ap()
    # [group, j, ch_in_group, i]
    t_all = nc.dram_tensor("t_all", [DPC // GT, L, GT, L], f16,
                           kind="ExternalInput").ap()
    e_all = nc.dram_tensor("e_all", [DPC // GE, L, GE, N_STATE], f16,
                           kind="ExternalInput").ap()
    # [group, p=(par,n), q_in_group, i]
    cb_all = nc.dram_tensor("cb_all", [64 // GQ, 128, GQ, L], f16,
                            kind="ExternalInput").ap()
    # scan multiplier: [p=(par,n), q, s]; 0 at s=0 (segment reset)
    p0 = nc.dram_tensor("p0", [128, 64, NCH], f16, kind="ExternalInput").ap()
    w2 = nc.dram_tensor("w2", [DPC, VOCAB], f16, kind="ExternalInput").ap()
    # [G, p=(ph,v), s, i*L]  (see host unpack)
    out_t = nc.dram_tensor("out_t", [BC // 8, 128, 2, 2 * L], f16,
                           kind="ExternalOutput").ap()

    with tile.TileContext(nc) as tc, ExitStack() as ctx:
        persist = ctx.enter_context(tc.tile_pool(name="persist", bufs=1))
        u_sb = persist.tile([128, DPC, BC], f16, name="u_sb")    # [j, d, t]
        # y: [j, tg, d, ti], t = 4*tg + ti
        y_sb = persist.tile([128, BC // 4, DPC, 4], f16, name="y_sb")
        # scan buffer [p=(par,n), b, q, s]; B writes S[c] into s=c+1,
        # slot 0 is zero; after the scan slot s=c holds hstart[c]
        x_sb = persist.tile([128, BATCH, 64, NCH], f16, name="x_sb")
        p0_sb = persist.tile([128, 64, NCH], f16, name="p0_sb")
        emb_sb = persist.tile([VOCAB, DPC], f16, name="emb_sb")
        w2_sb = persist.tile([DPC, VOCAB], f16, name="w2_sb")
        ident = persist.tile([128, 128], f16, name="ident")

        # one-hot staging pool opened first; DMAs ride the GpSimd ring
        ohp = ctx.enter_context(tc.tile_pool(name="ohp", bufs=7))
        oh_ts = {}

        def fetch_oh(g):
            oh_ts[g] = ohp.tile([VOCAB, GA, L], f16, name="oh_w", tag="oh_w")
            nc.gpsimd.dma_start(
                out=oh_ts[g], in_=onehotT[:, g * GA * L:(g + 1) * GA * L])

        # first bytes on the wire before any setup compute
        nc.sync.dma_start(out=emb_sb, in_=embs)
        for g in range(5):
            fetch_oh(g)

        # weight pools (Sync ring, independent of the one-hot stream)
        ewp = ctx.enter_context(tc.tile_pool(name="ewp", bufs=4))
        twp = ctx.enter_context(tc.tile_pool(name="twp", bufs=3))
        cbp = ctx.enter_context(tc.tile_pool(name="cbp", bufs=2))
        e_ws, t_ws, cb_ws = {}, {}, {}

        def fetch_e(g):
            e_ws[g] = ewp.tile([L, GE, N_STATE], f16, name="e_w", tag="e_w")
            nc.sync.dma_start(out=e_ws[g], in_=e_all[g])

        def fetch_t(g):
            t_ws[g] = twp.tile([L, GT, L], f16, name="t_w", tag="t_w")
            nc.sync.dma_start(out=t_ws[g], in_=t_all[g])

        def fetch_cb(g):
            cb_ws[g] = cbp.tile([128, GQ, L], f16, name="cb_w", tag="cb_w")
            nc.sync.dma_start(out=cb_ws[g], in_=cb_all[g])

        # weight prefetch: e first (phase B) + small tensors; t/cb wait
        # until phase A's one-hot stream is done with the HBM bandwidth
        for g in range(4):
            fetch_e(g)
        nc.sync.dma_start(out=p0_sb, in_=p0)
        nc.sync.dma_start(out=w2_sb, in_=w2)

        make_identity(nc, ident)
        nc.vector.memset(x_sb[:, :, :, 0], 0.0)

        def cp(i, out, in_):
            if i % 2 == 0:
                nc.vector.tensor_copy(out, in_)
            else:
                nc.scalar.copy(out, in_)

        def cp3(i, out, in_):
            # gpsimd cannot read PSUM; scalar-heavy split, DVE does scans
            if i % 6 < 5:
                nc.scalar.copy(out, in_)
            else:
                nc.vector.tensor_copy(out, in_)

        # ---- Phase A: embedding (one-hot @ emb slice) -> u_sb ----
        with tc.tile_pool(name="ps_a", bufs=3, space="PSUM") as ps_a:
            for g in range(BC // GA):                    # 32 groups of 8
                if g + 5 < BC // GA:
                    fetch_oh(g + 5)
                elif g == BC // GA - 5:
                    # one-hot stream finished: start the phase-D weights
                    fetch_t(0)
                    fetch_t(1)
                elif g == BC // GA - 4:
                    fetch_t(2)
                    fetch_cb(0)
                elif g == BC // GA - 3:
                    # t3 blocks on t0's pool buffer (freed early in D);
                    # must come after cb0 on the sync ring
                    fetch_t(3)
                oh = oh_ts[g]
                ups = ps_a.tile([128, 8, DPC], f32)      # 2 PSUM banks
                for i in range(8):
                    # one accumulation group per 2KB zero region
                    nc.tensor.matmul(ups[:, i, :],
                                     lhsT=oh[:, i, :], rhs=emb_sb,
                                     start=(i % 4 == 0), stop=(i % 4 == 3))
                t0 = g * GA
                # transposing drain: strided fp32 PSUM reads (cheap),
                # contiguous f16 writes into u_sb[:, d, t]
                cp(g, u_sb[:, :, t0:t0 + 8], ups.transpose([0, 2, 1]))

        # ---- Phase B: chunk-end states S[c] -> x_sb slots 1..31 ----
        # After each group's drain, scan that q-block for all batches
        # (DVE), so phase D's Cb matmuls never wait.
        with tc.tile_pool(name="ps_s", bufs=4, space="PSUM") as ps_s:
            for g in range(DPC // GE):                   # 8 groups of 16 ch
                if g >= 4:
                    fetch_e(g)
                e_w = e_ws[g]
                for k in range(GE // 4):                 # 2 q-pairs per bank
                    qb = (g * GE) // 2 + 2 * k           # first q of the bank
                    s_ps = ps_s.tile([128, 2, BATCH, NCH], f32)
                    for jq in range(2):
                        q = qb + jq
                        for par in range(2):
                            dl = 2 * q + par
                            # per-partition-range groups; the sim's group
                            # check mis-addresses split groups, skip it
                            nc.tensor.matmul(
                                s_ps[64 * par:64 * (par + 1), jq, :, :],
                                lhsT=e_w[:, dl - g * GE, :], rhs=u_sb[:, dl, :],
                                start=(jq == 0), stop=(jq == 1),
                                skip_group_check=True,
                                tile_position=(0, 64 * par) if par else None)
                    # drain S[c] -> x_sb[:, b, q, c+1] ((q2,b,c) -> (b,q2,s))
                    out_ap = x_sb[:, :, qb:qb + 2, 1:NCH].transpose([0, 2, 1, 3])
                    (nc.vector.tensor_copy if (k + g) % 4 == 3 else nc.scalar.copy)(out_ap, s_ps[:, :, :, 0:NCH - 1])
                if g % 2 == 1:
                    # scan the finished q-quarter for every batch, split
                    # DVE / GpSimd so neither engine becomes critical
                    q0 = (g // 2) * 16
                    mul = p0_sb[:, q0:q0 + 16, :].rearrange("p q s -> p (q s)")
                    for b in range(BATCH):
                        seg = x_sb[:, b, q0:q0 + 16, :].rearrange(
                            "p q s -> p (q s)")
                        nc.vector.tensor_tensor_scan(
                            out=seg, data0=mul, data1=seg,
                            initial=0.0, op0=mybir.AluOpType.mult,
                            op1=mybir.AluOpType.add)

        # ---- Phase D: per-channel y = T^T u (+) Cb^T hstart -> y_sb ----
        with tc.tile_pool(name="ps_y", bufs=4, space="PSUM") as ps_y:
            for q in range(64):
                if q % GQ == 0 and q > 0:
                    fetch_cb(q // GQ)
                if q % (GT // 2) == 0 and q >= 16:   # t0..t3 prefetched in A
                    fetch_t((2 * q) // GT)
                t_w = t_ws[(2 * q) // GT]
                cb_w = cb_ws[q // GQ]
                # one full 2KB bank per channel (row-offset tile_position
                # matmuls crash when a bank holds two accumulation slots),
                # but pack the q-pair into one 2-bank tile so the drain is
                # a single instruction
                y_ps = ps_y.tile([L, 2, 2 * BC], f32, name="y_ps",
                                 tag="y_ps")
                for par in range(2):
                    dl = 2 * q + par
                    nc.tensor.matmul(y_ps[:, par, 0:BC],
                                     lhsT=t_w[:, dl % GT, :],
                                     rhs=u_sb[:, dl, :],
                                     start=True, stop=False)
                for par in range(2):
                    h = x_sb[64 * par:64 * (par + 1), :, q, :]   # [64, b, c]
                    nc.tensor.matmul(
                        y_ps[:, par, 0:BC],
                        lhsT=cb_w[64 * par:64 * (par + 1), q % GQ, :],
                        rhs=h, start=False, stop=True,
                        tile_position=(64 * par, 0) if par else None)
                # [j, 2, (tg ti)] -> y_sb[:, tg, 2q:2q+2, ti]
                dcp = (nc.scalar.copy if (q < 32 or q % 2 == 0)
                       else nc.vector.tensor_copy)
                dcp(y_sb[:, :, 2 * q:2 * q + 2, :],
                    y_ps[:, :, 0:BC].rearrange("p c (g t) -> p c g t",
                                               g=BC // 4)
                    .transpose([0, 2, 1, 3]))

        # ---- Phase E: transpose y + output projection -> out_t ----
        with tc.tile_pool(name="ytp", bufs=2) as ytp, \
             tc.tile_pool(name="lop", bufs=2) as lop, \
             tc.tile_pool(name="ps_t", bufs=3, space="PSUM") as ps_t, \
             tc.tile_pool(name="ps_o", bufs=4, space="PSUM") as ps_o:
            for G in range(BC // 8):                     # 32 super-groups
                op = ps_o.tile([128, 2, 2 * L], f32)     # 4 logit tiles/bank
                tp = ps_t.tile([128, 8, 128], f16)       # one full bank
                yt = ytp.tile([128, 8, 128], f16)
                for s in range(2):
                    for i in range(4):
                        # tile t = 8G+4s+i = 4*(2G+s) + i
                        nc.tensor.matmul(
                            tp[:, 4 * s + i, :], lhsT=y_sb[:, 2 * G + s, :, i],
                            rhs=ident, is_transpose=True,
                            start=(4 * s + i == 0), stop=(4 * s + i == 7))
                nc.vector.tensor_copy(yt[:, 0:4, :], tp[:, 0:4, :])
                nc.scalar.copy(yt[:, 4:8, :], tp[:, 4:8, :])
                for s in range(2):
                    for ph in range(2):
                        nc.tensor.matmul(
                            op[64 * ph:64 * (ph + 1), s, :],
                            lhsT=w2_sb,
                            rhs=yt[:, 4 * s + 2 * ph:4 * s + 2 * ph + 2, :],
                            start=(s == 0), stop=(s == 1),
                            skip_group_check=True,
                            tile_position=(0, 64 * ph) if ph else None)
                lo = lop.tile([128, 2, 2 * L], f16)
                cp(G, lo, op)
                nc.sync.dma_start(out=out_t[G], in_=lo)


def _build_nc():
    import concourse.tile as tile
    from concourse import bacc, mybir
    from concourse.masks import make_identity

    nc = bacc.Bacc(trn_type="TRN2", target_bir_lowering=False, debug=False)
    _emit_kernel(nc, tile, mybir, make_identity)
    nc.compile()
    return nc


_NC_CACHE = None


def kernel(x, emb, log_neg_A, B, C, D, log_dt, W_out, b_out):
    global LAST_RESULTS, _NC_CACHE
    from concourse.bass_utils import run_bass_kernel_spmd

    x = np.asarray(x).astype(np.int64)
    emb = np.asarray(emb, np.float32)
    log_neg_A = np.asarray(log_neg_A, np.float32)
    B_in = np.asarray(B, np.float32)
    C = np.asarray(C, np.float32)
    D_in = np.asarray(D, np.float32)
    log_dt = np.asarray(log_dt, np.float32)
    W_out = np.asarray(W_out, np.float32)
    b_out = np.asarray(b_out, np.float32)

    T, E, Cb, P = _precompute_host(emb, log_neg_A, B_in, C, D_in, log_dt)

    # one-hot; b-major tiles: tok = (b*NCH + c)*L + j = row-major flat x
    toks = x.reshape(-1)
    onehotT = (np.arange(VOCAB)[:, None] == toks[None, :]).astype(np.float16)

    in_maps = []
    for core in range(NCORES):
        ds = slice(core * DPC, (core + 1) * DPC)
        # p0 layout [p=(par,n), q, s]: p = 64*par + n, d = 2*q + par; 0 at s=0
        Pc = P[ds].reshape(64, 2, N_STATE).transpose(1, 2, 0).reshape(128, 64)
        P0 = np.broadcast_to(Pc[:, :, None], (128, 64, NCH)).copy()
        P0[:, :, 0] = 0.0
        # t_all: [DPC,L,L] -> [DPC/GT, L, GT, L]
        Tc = np.ascontiguousarray(
            T[ds].reshape(DPC // GT, GT, L, L).transpose(0, 2, 1, 3))
        # e_all: [DPC,L,N] -> [DPC/GE, L, GE, N]
        Ec = np.ascontiguousarray(
            E[ds].reshape(DPC // GE, GE, L, N_STATE).transpose(0, 2, 1, 3))
        # cb_all: [DPC,N,L] -> pair-pack [64, 128=(par,n), L] -> groups of GQ
        Cbp = Cb[ds].reshape(64, 2 * N_STATE, L)   # [q, (par,n), L]
        Cbc = np.ascontiguousarray(
            Cbp.reshape(64 // GQ, GQ, 128, L).transpose(0, 2, 1, 3))
        in_maps.append({
            "onehot_t": onehotT,
            "emb_s": np.ascontiguousarray(emb[:, ds]).astype(np.float16),
            "t_all": Tc,
            "e_all": Ec,
            "cb_all": Cbc,
            "p0": P0.astype(np.float16),
            "w2": np.ascontiguousarray(W_out[ds]).astype(np.float16),
        })

    if _NC_CACHE is None:
        _NC_CACHE = _build_nc()
    nc = _NC_CACHE

    trace = bool(int(os.environ.get("BASS_TRACE", "0") or "0"))
    LAST_RESULTS = run_bass_kernel_spmd(
        nc, in_maps, core_ids=list(range(NCORES)), trace=trace)

    # out_t[G, 64*ph+v, s, 128*i+j] = logitsT[v, (8G+4s+2ph+i)*128 + j]
    logitsT = np.zeros((VOCAB, BC * L), np.float64)
    for r in LAST_RESULTS.results:
        o = r["out_t"].astype(np.float64).reshape(32, 2, 64, 2, 2, L)
        logitsT += o.transpose(2, 0, 3, 1, 4, 5).reshape(VOCAB, BC * L)
    # b-major tiles: col = ((b*NCH + c)*L + j)
    out = logitsT.T.reshape(BATCH, SEQ, VOCAB)
    return (out + b_out.astype(np.float64)).astype(np.float32)


# revision 18
# speedup vs baseline: 1.3971x; 1.1615x over previous
"""S4D "CopyingModel" Trainium2 Bass kernel (v6 — DMA rings + scan/drain
scheduling for continuous PE).

Math: logits = (s4d_scan(emb[x]) + emb[x]*D) @ W_out + b_out, with a
per-channel diagonal SSM (d_model=1024 channels, d_state=64).

Strategy (8 NeuronCores, channel-sharded: 128 channels per core).
Tiles are b-major (tile t = b*32 + c); every PE stream is contiguous:
  - u_sb [j, d, t]: per-channel matmul rhs contiguous
  - x_sb [p, b, q, s]: chunk-carry tensor_tensor_scan contiguous per b
  - y_sb [j, tg, d, ti] (t = 4*tg + ti): drain writes in runs of 4,
    transpose LDW reads at 8B stride (cheap on both sides)
v6 changes vs v5:
  - one-hot DMAs ride the GpSimd HWDGE ring, weights ride the Sync
    ring: the two streams no longer serialize behind each other
  - one-hot groups doubled (GA=16 -> 4KB per partition row), deep
    prefetch (bufs=6) starting before identity/memset setup
  - chunk-carry scans are q-block-major: one tensor_tensor_scan per
    phase-B channel group (all batches at once, p0 broadcast over b),
    so phase D's Cb matmuls never wait on a late scan
  - B/D PSUM drains on Scalar+GpSimd (DVE stays free for scans);
    A/E drains on Vector+Scalar
"""

import os
from contextlib import ExitStack

import numpy as np

BATCH = 8
SEQ = 4096
D_MODEL = 1024
N_STATE = 64
VOCAB = 64
L = 128                   # chunk length
NCH = SEQ // L            # 32 chunks
NCORES = 8
DPC = D_MODEL // NCORES   # 128 channels per core
BC = NCH * BATCH          # 256 token tiles; tile t = b*NCH + c

GA = 8    # one-hot tiles per DMA (phase A)
GE = 16   # E channels per DMA
GT = 8    # T channels per DMA
GQ = 16   # Cb channel-pairs per DMA

LAST_RESULTS = None       # BassKernelResults of the most recent run


def _precompute_host(emb, log_neg_A, Bmat, C, Dvec, log_dt):
    """Float64 host precompute of all device operands."""
    dt = np.exp(log_dt.astype(np.float64))                    # (D,)
    A = -np.exp(log_neg_A.astype(np.float64))                 # (D,N)
    dA = np.exp(dt[:, None] * A)                              # (D,N)
    dB = (dA - 1.0) / A * Bmat.astype(np.float64)             # (D,N)
    w = C.astype(np.float64) * dB                             # (D,N)

    # dApow[d,n,k] = dA^k, k=0..L-1
    dApow = np.ones((D_MODEL, N_STATE, L))
    np.cumprod(np.broadcast_to(dA[:, :, None], (D_MODEL, N_STATE, L - 1)),
               axis=2, out=dApow[:, :, 1:])
    K = np.einsum("dn,dnk->dk", w, dApow)                     # (D,L)
    K[:, 0] += Dvec.astype(np.float64)                        # fold skip

    # Toeplitz lhsT: T[d][j,i] = K[d, i-j] for i>=j
    T = np.zeros((D_MODEL, L, L), np.float32)
    Kf = K.astype(np.float32)
    for k in range(L):
        idx = np.arange(L - k)
        T[:, idx, idx + k] = Kf[:, k][:, None]

    # E lhsT [d, j, n] = dA^(L-1-j) * dB
    E = (dApow[:, :, ::-1] * dB[:, :, None]).transpose(0, 2, 1)  # (D,L,N)
    # Cb lhsT [d, n, i] = C * dA^(i+1)
    dApow1 = dApow * dA[:, :, None]
    Cb = C.astype(np.float64)[:, :, None] * dApow1               # (D,N,L)
    P = dApow1[:, :, L - 1]                                      # dA^L (D,N)
    return (T.astype(np.float16), E.astype(np.float16),
            Cb.astype(np.float16), P)


def _emit_kernel(nc, tile, mybir, make_identity):
    f16 = mybir.dt.float16
    f32 = mybir.dt.float32

    onehotT = nc.dram_tensor("onehot_t", [VOCAB, BC * L], f16,
                             kind="ExternalInput").ap()
    embs = nc.dram_tensor("emb_s", [VOCAB, DPC], f16, kind="ExternalInput").ap()
    # [group, j, ch_in_group, i]
    t_all = nc.dram_tensor("t_all", [DPC // GT, L, GT, L], f16,
                           kind="ExternalInput").ap()
    e_all = nc.dram_tensor("e_all", [DPC // GE, L, GE, N_STATE], f16,
                           kind="ExternalInput").ap()
    # [group, p=(par,n), q_in_group, i]
    cb_all = nc.dram_tensor("cb_all", [64 // GQ, 128, GQ, L], f16,
                            kind="ExternalInput").ap()
    # scan multiplier: [p=(par,n), q, s]; 0 at s=0 (segment reset)
    p0 = nc.dram_tensor("p0", [128, 64, NCH], f16, kind="ExternalInput").ap()
    w2 = nc.dram_tensor("w2", [DPC, VOCAB], f16, kind="ExternalInput").ap()
    # [G, p=(ph,v), s, i*L]  (see host unpack)
    out_t = nc.dram_tensor("out_t", [BC // 8, 128, 2, 2 * L], f16,
                           kind="ExternalOutput").ap()

    with tile.TileContext(nc) as tc, ExitStack() as ctx:
        persist = ctx.enter_context(tc.tile_pool(name="persist", bufs=1))
        u_sb = persist.tile([128, DPC, BC], f16, name="u_sb")    # [j, d, t]
        # y: [j, tg, d, ti], t = 4*tg + ti
        y_sb = persist.tile([128, BC // 4, DPC, 4], f16, name="y_sb")
        # scan buffer [p=(par,n), b, q, s]; B writes S[c] into s=c+1,
        # slot 0 is zero; after the scan slot s=c holds hstart[c]
        x_sb = persist.tile([128, BATCH, 64, NCH], f16, name="x_sb")
        p0_sb = persist.tile([128, 64, NCH], f16, name="p0_sb")
        emb_sb = persist.tile([VOCAB, DPC], f16, name="emb_sb")
        w2_sb = persist.tile([DPC, VOCAB], f16, name="w2_sb")
        ident = persist.tile([128, 128], f16, name="ident")

        # one-hot staging pool opened first; DMAs ride the GpSimd ring
        ohp = ctx.enter_context(tc.tile_pool(name="ohp", bufs=7))
        oh_ts = {}

        def fetch_oh(g):
            oh_ts[g] = ohp.tile([VOCAB, GA, L], f16, name="oh_w", tag="oh_w")
            nc.gpsimd.dma_start(
                out=oh_ts[g], in_=onehotT[:, g * GA * L:(g + 1) * GA * L])

        # first bytes on the wire before any setup compute
        nc.sync.dma_start(out=emb_sb, in_=embs)
        for g in range(5):
            fetch_oh(g)

        # weight pools (Sync ring, independent of the one-hot stream)
        ewp = ctx.enter_context(tc.tile_pool(name="ewp", bufs=4))
        twp = ctx.enter_context(tc.tile_pool(name="twp", bufs=3))
        cbp = ctx.enter_context(tc.tile_pool(name="cbp", bufs=2))
        e_ws, t_ws, cb_ws = {}, {}, {}

        def fetch_e(g):
            e_ws[g] = ewp.tile([L, GE, N_STATE], f16, name="e_w", tag="e_w")
            nc.sync.dma_start(out=e_ws[g], in_=e_all[g])

        def fetch_t(g):
            t_ws[g] = twp.tile([L, GT, L], f16, name="t_w", tag="t_w")
            nc.sync.dma_start(out=t_ws[g], in_=t_all[g])

        def fetch_cb(g):
            cb_ws[g] = cbp.tile([128, GQ, L], f16, name="cb_w", tag="cb_w")
            nc.sync.dma_start(out=cb_ws[g], in_=cb_all[g])

        # weight prefetch: e first (phase B), then t (phase D), then cb
        for g in range(4):
            fetch_e(g)
        nc.sync.dma_start(out=p0_sb, in_=p0)
        nc.sync.dma_start(out=w2_sb, in_=w2)
        for g in range(2):
            fetch_t(g)
        fetch_cb(0)

        make_identity(nc, ident)
        nc.vector.memset(x_sb[:, :, :, 0], 0.0)

        def cp(i, out, in_):
            if i % 2 == 0:
                nc.vector.tensor_copy(out, in_)
            else:
                nc.scalar.copy(out, in_)

        def cp3(i, out, in_):
            # gpsimd cannot read PSUM; scalar-heavy split, DVE does scans
            if i % 6 < 5:
                nc.scalar.copy(out, in_)
            else:
                nc.vector.tensor_copy(out, in_)

        # ---- Phase A: embedding (one-hot @ emb slice) -> u_sb ----
        with tc.tile_pool(name="ps_a", bufs=3, space="PSUM") as ps_a:
            for g in range(BC // GA):                    # 32 groups of 8
                if g + 5 < BC // GA:
                    fetch_oh(g + 5)
                elif g == BC // GA - 5:
                    # tail of the fetch schedule: remaining t groups
                    fetch_t(2)
                    fetch_t(3)
                oh = oh_ts[g]
                ups = ps_a.tile([128, 8, DPC], f32)      # 2 PSUM banks
                for i in range(8):
                    # one accumulation group per 2KB zero region
                    nc.tensor.matmul(ups[:, i, :],
                                     lhsT=oh[:, i, :], rhs=emb_sb,
                                     start=(i % 4 == 0), stop=(i % 4 == 3))
                t0 = g * GA
                # transposing drain: strided fp32 PSUM reads (cheap),
                # contiguous f16 writes into u_sb[:, d, t]
                cp(g, u_sb[:, :, t0:t0 + 8], ups.transpose([0, 2, 1]))

        # ---- Phase B: chunk-end states S[c] -> x_sb slots 1..31 ----
        # After each group's drain, scan that q-block for all batches
        # (DVE), so phase D's Cb matmuls never wait.
        with tc.tile_pool(name="ps_s", bufs=4, space="PSUM") as ps_s:
            for g in range(DPC // GE):                   # 8 groups of 16 ch
                if g >= 4:
                    fetch_e(g)
                e_w = e_ws[g]
                for k in range(GE // 4):                 # 2 q-pairs per bank
                    qb = (g * GE) // 2 + 2 * k           # first q of the bank
                    s_ps = ps_s.tile([128, 2, BATCH, NCH], f32)
                    for jq in range(2):
                        q = qb + jq
                        for par in range(2):
                            dl = 2 * q + par
                            # per-partition-range groups; the sim's group
                            # check mis-addresses split groups, skip it
                            nc.tensor.matmul(
                                s_ps[64 * par:64 * (par + 1), jq, :, :],
                                lhsT=e_w[:, dl - g * GE, :], rhs=u_sb[:, dl, :],
                                start=(jq == 0), stop=(jq == 1),
                                skip_group_check=True,
                                tile_position=(0, 64 * par) if par else None)
                    # drain S[c] -> x_sb[:, b, q, c+1] ((q2,b,c) -> (b,q2,s))
                    out_ap = x_sb[:, :, qb:qb + 2, 1:NCH].transpose([0, 2, 1, 3])
                    (nc.vector.tensor_copy if (k + g) % 4 == 3 else nc.scalar.copy)(out_ap, s_ps[:, :, :, 0:NCH - 1])
                if g % 2 == 1:
                    # scan the finished q-quarter for every batch, split
                    # DVE / GpSimd so neither engine becomes critical
                    q0 = (g // 2) * 16
                    mul = p0_sb[:, q0:q0 + 16, :].rearrange("p q s -> p (q s)")
                    for b in range(BATCH):
                        seg = x_sb[:, b, q0:q0 + 16, :].rearrange(
                            "p q s -> p (q s)")
                        nc.vector.tensor_tensor_scan(
                            out=seg, data0=mul, data1=seg,
                            initial=0.0, op0=mybir.AluOpType.mult,
                            op1=mybir.AluOpType.add)

        # ---- Phase D: per-channel y = T^T u (+) Cb^T hstart -> y_sb ----
        with tc.tile_pool(name="ps_y", bufs=4, space="PSUM") as ps_y:
            for q in range(64):
                if q % GQ == 0 and q > 0:
                    fetch_cb(q // GQ)
                if q % (GT // 2) == 0 and q >= 16:   # t0..t3 prefetched in A
                    fetch_t((2 * q) // GT)
                t_w = t_ws[(2 * q) // GT]
                cb_w = cb_ws[q // GQ]
                # one full 2KB bank per channel (row-offset tile_position
                # matmuls crash when a bank holds two accumulation slots),
                # but pack the q-pair into one 2-bank tile so the drain is
                # a single instruction
                y_ps = ps_y.tile([L, 2, 2 * BC], f32, name="y_ps",
                                 tag="y_ps")
                for par in range(2):
                    dl = 2 * q + par
                    nc.tensor.matmul(y_ps[:, par, 0:BC],
                                     lhsT=t_w[:, dl % GT, :],
                                     rhs=u_sb[:, dl, :],
                                     start=True, stop=False)
                for par in range(2):
                    h = x_sb[64 * par:64 * (par + 1), :, q, :]   # [64, b, c]
                    nc.tensor.matmul(
                        y_ps[:, par, 0:BC],
                        lhsT=cb_w[64 * par:64 * (par + 1), q % GQ, :],
                        rhs=h, start=False, stop=True,
                        tile_position=(64 * par, 0) if par else None)
                # [j, 2, (tg ti)] -> y_sb[:, tg, 2q:2q+2, ti]
                dcp = (nc.scalar.copy if (q < 32 or q % 2 == 0)
                       else nc.vector.tensor_copy)
                dcp(y_sb[:, :, 2 * q:2 * q + 2, :],
                    y_ps[:, :, 0:BC].rearrange("p c (g t) -> p c g t",
                                               g=BC // 4)
                    .transpose([0, 2, 1, 3]))

        # ---- Phase E: transpose y + output projection -> out_t ----
        with tc.tile_pool(name="ytp", bufs=2) as ytp, \
             tc.tile_pool(name="lop", bufs=2) as lop, \
             tc.tile_pool(name="ps_t", bufs=3, space="PSUM") as ps_t, \
             tc.tile_pool(name="ps_o", bufs=4, space="PSUM") as ps_o:
            for G in range(BC // 8):                     # 32 super-groups
                op = ps_o.tile([128, 2, 2 * L], f32)     # 4 logit tiles/bank
                tp = ps_t.tile([128, 8, 128], f16)       # one full bank
                yt = ytp.tile([128, 8, 128], f16)
                for s in range(2):
                    for i in range(4):
                        # tile t = 8G+4s+i = 4*(2G+s) + i
                        nc.tensor.matmul(
                            tp[:, 4 * s + i, :], lhsT=y_sb[:, 2 * G + s, :, i],
                            rhs=ident, is_transpose=True,
                            start=(4 * s + i == 0), stop=(4 * s + i == 7))
                nc.vector.tensor_copy(yt[:, 0:4, :], tp[:, 0:4, :])
                nc.scalar.copy(yt[:, 4:8, :], tp[:, 4:8, :])
                for s in range(2):
                    for ph in range(2):
                        nc.tensor.matmul(
                            op[64 * ph:64 * (ph + 1), s, :],
                            lhsT=w2_sb,
                            rhs=yt[:, 4 * s + 2 * ph:4 * s + 2 * ph + 2, :],
                            start=(s == 0), stop=(s == 1),
                            skip_group_check=True,
                            tile_position=(0, 64 * ph) if ph else None)
                lo = lop.tile([128, 2, 2 * L], f16)
                cp(G, lo, op)
                nc.sync.dma_start(out=out_t[G], in_=lo)


def _build_nc():
    import concourse.tile as tile
    from concourse import bacc, mybir
    from concourse.masks import make_identity

    nc = bacc.Bacc(trn_type="TRN2", target_bir_lowering=False, debug=False)
    _emit_kernel(nc, tile, mybir, make_identity)
    nc.compile()
    return nc


_NC_CACHE = None


def kernel(x, emb, log_neg_A, B, C, D, log_dt, W_out, b_out):
    global LAST_RESULTS, _NC_CACHE
    from concourse.bass_utils import run_bass_kernel_spmd

    x = np.asarray(x).astype(np.int64)
    emb = np.asarray(emb, np.float32)
    log_neg_A = np.asarray(log_neg_A, np.float32)
    B_in = np.asarray(B, np.float32)
    C = np.asarray(C, np.float32)
    D_in = np.asarray(D, np.float32)
    log_dt = np.asarray(log_dt, np.float32)
    W_out = np.asarray(W_out, np.float32)
    b_out = np.asarray(b_out, np.float32)

    T, E, Cb, P = _precompute_host(emb, log_neg_A, B_in, C, D_in, log_dt)

    # one-hot; b-major tiles: tok = (b*NCH + c)*L + j = row-major flat x
    toks = x.reshape(-1)
    onehotT = (np.arange(VOCAB)[:, None] == toks[None, :]).astype(np.float16)

    in_maps = []
    for core in range(NCORES):
        ds = slice(core * DPC, (core + 1) * DPC)
        # p0 layout [p=(par,n), q, s]: p = 64*par + n, d = 2*q + par; 0 at s=0
        Pc = P[ds].reshape(64, 2, N_STATE).transpose(1, 2, 0).reshape(128, 64)
        P0 = np.broadcast_to(Pc[:, :, None], (128, 64, NCH)).copy()
        P0[:, :, 0] = 0.0
        # t_all: [DPC,L,L] -> [DPC/GT, L, GT, L]
        Tc = np.ascontiguousarray(
            T[ds].reshape(DPC // GT, GT, L, L).transpose(0, 2, 1, 3))
        # e_all: [DPC,L,N] -> [DPC/GE, L, GE, N]
        Ec = np.ascontiguousarray(
            E[ds].reshape(DPC // GE, GE, L, N_STATE).transpose(0, 2, 1, 3))
        # cb_all: [DPC,N,L] -> pair-pack [64, 128=(par,n), L] -> groups of GQ
        Cbp = Cb[ds].reshape(64, 2 * N_STATE, L)   # [q, (par,n), L]
        Cbc = np.ascontiguousarray(
            Cbp.reshape(64 // GQ, GQ, 128, L).transpose(0, 2, 1, 3))
        in_maps.append({
            "onehot_t": onehotT,
            "emb_s": np.ascontiguousarray(emb[:, ds]).astype(np.float16),
            "t_all": Tc,
            "e_all": Ec,
            "cb_all": Cbc,
            "p0": P0.astype(np.float16),
            "w2": np.ascontiguousarray(W_out[ds]).astype(np.float16),
        })

    if _NC_CACHE is None:
        _NC_CACHE = _build_nc()
    nc = _NC_CACHE

    trace = bool(int(os.environ.get("BASS_TRACE", "0") or "0"))
    LAST_RESULTS = run_bass_kernel_spmd(
        nc, in_maps, core_ids=list(range(NCORES)), trace=trace)

    # out_t[G, 64*ph+v, s, 128*i+j] = logitsT[v, (8G+4s+2ph+i)*128 + j]
    logitsT = np.zeros((VOCAB, BC * L), np.float64)
    for r in LAST_RESULTS.results:
        o = r["out_t"].astype(np.float64).reshape(32, 2, 64, 2, 2, L)
        logitsT += o.transpose(2, 0, 3, 1, 4, 5).reshape(VOCAB, BC * L)
    # b-major tiles: col = ((b*NCH + c)*L + j)
    out = logitsT.T.reshape(BATCH, SEQ, VOCAB)
    return (out + b_out.astype(np.float64)).astype(np.float32)
